# revision 1
# baseline (speedup 1.0000x reference)
"""DeepSetPred Trainium2 kernel: 3-layer token encoder MLP + segment-sum +
predictor MLP on 8 NeuronCores, with ZERO collectives.

Sharding: the host cuts the (sorted-by-segment) token axis at segment
boundaries, so every segment belongs to exactly one core. Each shard is
padded to a common length with tokens whose one-hot selector row is all
zero (they flow through the encoder but contribute nothing to any segment).
Each core therefore computes the complete segment sums for its own
contiguous range of <=32 segments, runs the predictor on just those rows,
and writes its private slice of the output; the host concatenates.

Layout: tokens on the matmul free dim (features on partitions), fp16
encoder matmuls (2-byte fast weight loads, fp32 PSUM accumulation,
~7e-4 rel err), per-feature bias+tanh fused on the ScalarEngine, the
ragged segment-sum as a one-hot stationary matmul accumulating into a
persistent PSUM bank with the n_s*b3 bias folded in as one K=1 fp32
matmul, and an fp32r predictor.
"""

import numpy as np

import concourse.mybir as mybir
import concourse.tile as tile
from concourse import bacc
from concourse import bass_utils
from concourse.masks import make_identity

# Problem shapes (hardcoded per contract).
T, E, H, C, O = 131072, 256, 512, 256, 32
S = 128            # num segments
N_CORES = 8
TOK = 512          # tokens per inner chunk
G = 1              # chunks per super-chunk (DMA batching granularity)
SCTOK = G * TOK    # 1024
MIN_SLOTS = 32     # baseline segments-per-core capacity
F32 = mybir.dt.float32
F32R = mybir.dt.float32r
F16 = mybir.dt.float16

_CACHE = {}


def _mm(nc, out, lhsT, rhs, start, stop, skip=False):
    nc.tensor.matmul(out, lhsT, rhs,
                     start=start, stop=stop, skip_group_check=skip)


def _build_nc(t_sh, SLOTS):
    assert t_sh % 128 == 0

    nc = bacc.Bacc("TRN2", target_bir_lowering=False, debug=False,
                   num_devices=N_CORES)

    xt_d = nc.dram_tensor("xt", [E, t_sh], F16, kind="ExternalInput")
    sel_d = nc.dram_tensor("sel", [t_sh, SLOTS], F16, kind="ExternalInput")
    cnt_d = nc.dram_tensor("cnt", [1, SLOTS], F32, kind="ExternalInput")
    w1_d = nc.dram_tensor("w1", [E, H], F16, kind="ExternalInput")
    w2_d = nc.dram_tensor("w2", [H, H], F16, kind="ExternalInput")
    w3_d = nc.dram_tensor("w3", [H, C], F16, kind="ExternalInput")
    b1_d = nc.dram_tensor("b1", [H // 128, 128], F32, kind="ExternalInput")
    b2_d = nc.dram_tensor("b2", [H // 128, 128], F32, kind="ExternalInput")
    b3_d = nc.dram_tensor("b3", [1, C], F32, kind="ExternalInput")
    p1_d = nc.dram_tensor("p1", [C, H], F32R, kind="ExternalInput")
    p2_d = nc.dram_tensor("p2", [H, H], F32R, kind="ExternalInput")
    p3_d = nc.dram_tensor("p3", [H, O], F32R, kind="ExternalInput")
    pb1_d = nc.dram_tensor("pb1", [H // 128, 128], F32, kind="ExternalInput")
    pb2_d = nc.dram_tensor("pb2", [H // 128, 128], F32, kind="ExternalInput")
    pb3_d = nc.dram_tensor("pb3", [1, O], F32, kind="ExternalInput")
    out_d = nc.dram_tensor("pred", [SLOTS, O], F32, kind="ExternalOutput")

    EC = E // 128   # 2
    HC = H // 128   # 4
    CC = C // 128   # 2
    TT = TOK // 128  # 4 token sub-tiles per chunk

    with tile.TileContext(nc) as tc:
        with tc.tile_pool(name="wts", bufs=1) as wp, \
             tc.tile_pool(name="xt", bufs=3) as xtp, \
             tc.tile_pool(name="sel", bufs=3) as selp, \
             tc.tile_pool(name="act", bufs=4) as actp, \
             tc.tile_pool(name="small", bufs=1) as smp, \
             tc.tile_pool(name="ps", bufs=2, space="PSUM") as psp, \
             tc.tile_pool(name="psacc", bufs=1, space="PSUM") as psa:

            # warm the ACT tanh table before the scalar queue fills with DMAs
            warm_sb = smp.tile([1, 1], F32, tag="warm", name="warm")
            nc.gpsimd.memset(warm_sb[:], 0.0)
            warm_o = smp.tile([1, 1], F32, tag="warmo", name="warmo")
            nc.scalar.activation(warm_o[:], warm_sb[:],
                                 mybir.ActivationFunctionType.Tanh)

            # ---- resident weights (one batched HWDGE DMA per matrix;
            # encoder weights on the scalar ring, predictor weights on
            # gpsimd so the ACT queue stays clear for tanh) ----
            w1_t = wp.tile([128, EC, HC, 128], F16, tag="w1", name="w1t")
            nc.scalar.dma_start(
                w1_t[:], w1_d.ap().rearrange("(e p) (h q) -> p e h q",
                                             p=128, q=128))
            w1_sb = [[w1_t[:, e, h, :] for h in range(HC)] for e in range(EC)]
            w2_t = wp.tile([128, HC, HC, 128], F16, tag="w2", name="w2t")
            nc.scalar.dma_start(
                w2_t[:], w2_d.ap().rearrange("(k p) (h q) -> p k h q",
                                             p=128, q=128))
            w2_sb = [[w2_t[:, k, h, :] for h in range(HC)] for k in range(HC)]
            w3_t = wp.tile([128, HC, C], F16, tag="w3", name="w3t")
            nc.scalar.dma_start(
                w3_t[:], w3_d.ap().rearrange("(k p) c -> p k c", p=128))
            w3_sb = [w3_t[:, k, :] for k in range(HC)]
            # ---- biases / rows (gpsimd ring; tiny) ----
            b1_sb = smp.tile([128, HC], F32, tag="b1", name="b1")
            nc.gpsimd.dma_start(b1_sb[:], b1_d.ap().rearrange("h p -> p h"))
            b2_sb = smp.tile([128, HC], F32, tag="b2", name="b2")
            nc.gpsimd.dma_start(b2_sb[:], b2_d.ap().rearrange("h p -> p h"))
            pb1_sb = smp.tile([128, HC], F32, tag="pb1", name="pb1")
            nc.gpsimd.dma_start(pb1_sb[:], pb1_d.ap().rearrange("h p -> p h"))
            pb2_sb = smp.tile([128, HC], F32, tag="pb2", name="pb2")
            nc.gpsimd.dma_start(pb2_sb[:], pb2_d.ap().rearrange("h p -> p h"))
            b3row = smp.tile([1, C], F32, tag="b3row", name="b3row")
            nc.gpsimd.dma_start(b3row[:], b3_d.ap())
            pb3row = smp.tile([1, O], F32, tag="pb3row", name="pb3row")
            nc.gpsimd.dma_start(pb3row[:], pb3_d.ap())
            cntrow = smp.tile([1, SLOTS], F32, tag="cntrow", name="cntrow")
            nc.gpsimd.dma_start(cntrow[:], cnt_d.ap())
            ones1 = smp.tile([1, SLOTS], F32, tag="ones1", name="ones1")
            nc.gpsimd.memset(ones1[:], 1.0)
            ident = smp.tile([SLOTS, SLOTS], F32, tag="ident", name="ident")
            make_identity(nc, ident[:])

            p1_t = wp.tile([128, CC, HC, 128], F32R, tag="p1", name="p1t")
            nc.gpsimd.dma_start(
                p1_t[:], p1_d.ap().rearrange("(c p) (h q) -> p c h q",
                                             p=128, q=128))
            p1_sb = [[p1_t[:, c, h, :] for h in range(HC)] for c in range(CC)]
            p2_t = wp.tile([128, HC, HC, 128], F32R, tag="p2", name="p2t")
            nc.gpsimd.dma_start(
                p2_t[:], p2_d.ap().rearrange("(k p) (h q) -> p k h q",
                                             p=128, q=128))
            p2_sb = [[p2_t[:, k, h, :] for h in range(HC)] for k in range(HC)]
            p3_t = wp.tile([128, HC, O], F32R, tag="p3", name="p3t")
            nc.gpsimd.dma_start(
                p3_t[:], p3_d.ap().rearrange("(k p) o -> p k o", p=128))
            p3_sb = [p3_t[:, k, :] for k in range(HC)]

            # ---- persistent segment-sum accumulator enc[slot, c] ----
            enc_ps = psa.tile([SLOTS, C], F32, tag="encacc", name="encacc")
            # enc[slot, c] = counts[slot] * b3[c]  (K=1 fp32 matmul opens it)
            nc.tensor.matmul(enc_ps[:], cntrow[:], b3row[:],
                             start=True, stop=False, skip_group_check=True)

            # ---- main token loop, software-pipelined with a 1-chunk skew:
            # L1(i+1) is emitted before L2/L3/seg(i) so the PE's strict-FIFO
            # queue never head-of-line blocks on the tanh chain ----
            n_full = t_sh // TOK
            tail = t_sh - n_full * TOK
            chunks = [(i * TOK, TOK) for i in range(n_full)]
            if tail:
                chunks.append((n_full * TOK, tail))

            def load_and_l1(base, tok):
                xt_t = xtp.tile([128, EC, tok], F16, tag="xt", name="xt",
                                padded_shape=[128, EC, TOK])
                nc.sync.dma_start(
                    xt_t[:],
                    xt_d.ap()[:, base:base + tok]
                        .rearrange("(e p) t -> p e t", p=128))
                sel_t = selp.tile([128, tok // 128, SLOTS], F16, tag="sel",
                                  name="sel", padded_shape=[128, TT, SLOTS])
                nc.sync.dma_start(
                    sel_t[:],
                    sel_d.ap()[base:base + tok, :]
                         .rearrange("(q p) s -> p q s", p=128))
                h1_t = actp.tile([128, HC, tok], F16, tag="h1", name="h1",
                                 bufs=5, padded_shape=[128, HC, TOK])
                for h in range(HC):
                    ps1 = psp.tile([128, tok], F32, tag="mm", name="mm",
                                   bufs=4, padded_shape=[128, TOK])
                    for e in range(EC):
                        _mm(nc, ps1[:], w1_sb[e][h], xt_t[:, e, :],
                            start=(e == 0), stop=(e == EC - 1), skip=True)
                    nc.scalar.activation(h1_t[:, h, :], ps1[:],
                                         mybir.ActivationFunctionType.Tanh,
                                         bias=b1_sb[:, h:h + 1])
                return sel_t, h1_t

            def l2_l3_seg(sel_t, h1_t, tok, is_last):
                tt = tok // 128
                h2_t = actp.tile([128, HC, tok], F16, tag="h2", name="h2",
                                 padded_shape=[128, HC, TOK])
                for h in range(HC):
                    ps2 = psp.tile([128, tok], F32, tag="mm", name="mm",
                                   bufs=4, padded_shape=[128, TOK])
                    for k in range(HC):
                        _mm(nc, ps2[:], w2_sb[k][h], h1_t[:, k, :],
                            start=(k == 0), stop=(k == HC - 1), skip=True)
                    nc.scalar.activation(h2_t[:, h, :], ps2[:],
                                         mybir.ActivationFunctionType.Tanh,
                                         bias=b2_sb[:, h:h + 1])
                te_sb = actp.tile([128, tt, C], F16, tag="te", name="te",
                                  padded_shape=[128, TT, C])
                for t in range(tt):
                    ps3 = psp.tile([128, C], F32, tag="l3", name="l3", bufs=3)
                    for k in range(HC):
                        _mm(nc, ps3[:], h2_t[:, k, t * 128:(t + 1) * 128],
                            w3_sb[k], start=(k == 0), stop=(k == HC - 1))
                    nc.vector.tensor_copy(te_sb[:, t, :], ps3[:])
                for t in range(tt):
                    last = is_last and (t == tt - 1)
                    _mm(nc, enc_ps[:], sel_t[:, t, :], te_sb[:, t, :],
                        start=False, stop=last, skip=True)

            pend = []
            for ci, (base, tok) in enumerate(chunks):
                pend.append(load_and_l1(base, tok) + (tok,))
                keep = 2 if ci < 2 else 1
                while len(pend) > keep:
                    l2_l3_seg(*pend.pop(0), is_last=False)
            while pend:
                args = pend.pop(0)
                l2_l3_seg(*args, is_last=(len(pend) == 0))

            # ---- predictor on this core's own <=SLOTS segment rows ----
            enc_sb = smp.tile([SLOTS, C], F32, tag="encsb", name="encsb")
            nc.vector.tensor_copy(enc_sb[:], enc_ps[:])
            encT_sb = smp.tile([128, CC, SLOTS], F32R, tag="encT", name="encT")
            for c in range(CC):
                pst = psp.tile([128, SLOTS], F32, tag="l3", name="pst", bufs=3)
                nc.tensor.transpose(pst[:], enc_sb[:, c * 128:(c + 1) * 128],
                                    ident[:])
                nc.vector.tensor_copy(encT_sb[:, c, :], pst[:])

            q1_sb = smp.tile([128, HC, SLOTS], F32R, tag="q1", name="q1")
            for h in range(HC):
                pp1 = psp.tile([128, SLOTS], F32, tag="mm", name="pp1", bufs=4)
                for c in range(CC):
                    _mm(nc, pp1[:], p1_sb[c][h], encT_sb[:, c, :],
                        start=(c == 0), stop=(c == CC - 1))
                nc.scalar.activation(q1_sb[:, h, :], pp1[:],
                                     mybir.ActivationFunctionType.Tanh,
                                     bias=pb1_sb[:, h:h + 1])
            q2_sb = smp.tile([128, HC, SLOTS], F32R, tag="q2", name="q2")
            for h in range(HC):
                pp2 = psp.tile([128, SLOTS], F32, tag="mm", name="pp2", bufs=4)
                for k in range(HC):
                    _mm(nc, pp2[:], p2_sb[k][h], q1_sb[:, k, :],
                        start=(k == 0), stop=(k == HC - 1))
                nc.scalar.activation(q2_sb[:, h, :], pp2[:],
                                     mybir.ActivationFunctionType.Tanh,
                                     bias=pb2_sb[:, h:h + 1])

            # final: pred[slot, o] = q2.T @ P3 + pb3
            ppo = psp.tile([SLOTS, O], F32, tag="l3", name="ppo", bufs=3)
            nc.tensor.matmul(ppo[:], ones1[:], pb3row[:],
                             start=True, stop=False, skip_group_check=True)
            for k in range(HC):
                _mm(nc, ppo[:], q2_sb[:, k, :], p3_sb[k],
                    start=False, stop=(k == HC - 1), skip=True)
            pred_sb = smp.tile([SLOTS, O], F32, tag="pred", name="predsb")
            nc.vector.tensor_copy(pred_sb[:], ppo[:])
            nc.sync.dma_start(out_d.ap(), pred_sb[:])

    nc.compile()
    return nc


def kernel(words, seg_ids, W1, b1, W2, b2, W3, b3,
           P1, pb1, P2, pb2, P3, pb3, batch_size, alpha_iter, **_):
    words = np.asarray(words, dtype=np.float32)
    seg_ids = np.asarray(seg_ids).astype(np.int64)
    assert words.shape == (T, E), words.shape
    bs, ai = int(batch_size), int(alpha_iter)

    # --- host-side index prep: cut the sorted token axis at segment
    # boundaries so each core owns whole segments ---
    counts = np.bincount(seg_ids, minlength=S)[:S]
    starts = np.concatenate([[0], np.cumsum(counts)])   # [S+1]
    cuts = [0]
    for c in range(1, N_CORES):
        tgt = c * T // N_CORES
        j = int(np.searchsorted(starts, tgt, side="left"))
        if j > 0 and tgt - starts[j - 1] < starts[j] - tgt:
            j -= 1
        cuts.append(int(starts[j]))
    cuts.append(T)
    lens = np.diff(cuts)
    t_sh = int(np.ceil(lens.max() / 128) * 128)

    # contiguous segment range owned by each core (covers all of [0, S));
    # empty shards inherit the following shard's start so ranges stay
    # monotone and collectively exhaustive
    seg_lo = [0] * N_CORES
    for c in range(N_CORES - 1, 0, -1):
        if lens[c] > 0:
            seg_lo[c] = int(seg_ids[cuts[c]])
        else:
            seg_lo[c] = S if c == N_CORES - 1 else seg_lo[c + 1]
    seg_hi = seg_lo[1:] + [S]
    slots_needed = max(seg_hi[c] - seg_lo[c] for c in range(N_CORES))
    SLOTS = min(128, max(MIN_SLOTS, ((slots_needed + 31) // 32) * 32))
    assert slots_needed <= SLOTS, (seg_lo, seg_hi)
    assert bs * ai == S

    xt = np.ascontiguousarray(words.T.astype(np.float16))    # [E, T] fp16

    key = ("nc", t_sh, SLOTS)
    if key not in _CACHE:
        _CACHE[key] = _build_nc(t_sh, SLOTS)
    nc = _CACHE[key]

    common = {
        "w1": np.ascontiguousarray(W1, dtype=np.float16),
        "w2": np.ascontiguousarray(W2, dtype=np.float16),
        "w3": np.ascontiguousarray(W3, dtype=np.float16),
        "b1": np.ascontiguousarray(b1, dtype=np.float32).reshape(H // 128, 128),
        "b2": np.ascontiguousarray(b2, dtype=np.float32).reshape(H // 128, 128),
        "b3": np.ascontiguousarray(b3, dtype=np.float32).reshape(1, C),
        "p1": np.ascontiguousarray(P1, dtype=np.float32),
        "p2": np.ascontiguousarray(P2, dtype=np.float32),
        "p3": np.ascontiguousarray(P3, dtype=np.float32),
        "pb1": np.ascontiguousarray(pb1, dtype=np.float32).reshape(H // 128, 128),
        "pb2": np.ascontiguousarray(pb2, dtype=np.float32).reshape(H // 128, 128),
        "pb3": np.ascontiguousarray(pb3, dtype=np.float32).reshape(1, O),
    }
    in_maps = []
    for c in range(N_CORES):
        lo, hi = cuts[c], cuts[c + 1]
        n = hi - lo
        xt_c = np.zeros((E, t_sh), dtype=np.float16)
        xt_c[:, :n] = xt[:, lo:hi]
        sel_c = np.zeros((t_sh, SLOTS), dtype=np.float16)
        sel_c[:n, :] = (seg_ids[lo:hi, None] ==
                        (seg_lo[c] + np.arange(SLOTS))[None, :])
        cnt_c = np.zeros((1, SLOTS), dtype=np.float32)
        nseg = seg_hi[c] - seg_lo[c]
        cnt_c[0, :nseg] = counts[seg_lo[c]:seg_hi[c]]
        in_maps.append({
            **common,
            "xt": xt_c,
            "sel": sel_c,
            "cnt": cnt_c,
        })

    global _LAST_IN_MAPS
    _LAST_IN_MAPS = in_maps
    res = bass_utils.run_bass_kernel_spmd(nc, in_maps,
                                          core_ids=list(range(N_CORES)))
    pred = np.zeros((S, O), dtype=np.float32)
    for c in range(N_CORES):
        nseg = seg_hi[c] - seg_lo[c]
        if nseg > 0:
            pred[seg_lo[c]:seg_hi[c]] = res.results[c]["pred"][:nseg]
    return pred.reshape(bs, ai, O).astype(np.float32)


_LAST_IN_MAPS = None



# revision 4
# speedup vs baseline: 1.2812x; 1.2812x over previous
"""DeepSetPred Trainium2 kernel: 3-layer token encoder MLP + segment-sum +
predictor MLP on 8 NeuronCores, ZERO collectives.

Key structural wins over the naive mapping:

1. The third encoder layer is linear, so it commutes with the segment-sum:
       enc = segsum(h2 @ W3 + b3) = segsum(h2) @ W3 + counts * b3
   The per-token L3 matmul (25% of encoder FLOPs) and the one-hot
   segment-matmul collapse into one tiny [S,H]x[H,C] matmul after pooling.

2. segsum(h2) is computed without materializing anything token-major:
   - The L2 tanh on the ScalarEngine emits accum_out = sum over the whole
     chunk's tokens per H-partition (free side output).
   - Tokens are sorted by segment, so a 1024-token chunk contains at most
     two segment boundaries.  Two DVE scalar_tensor_tensor ops per
     (h-tile, chunk) compute head sums  sum_{t < b} h2[:, t]  via the fused
     (iota is_lt b) mult h2 -> accum_out form, with b per chunk as DATA
     (uniform instruction stream across all 8 cores -> one NEFF).
   - hsegT[h, s] = sum_c head1*(M1-M2) + head2*(M2-M3) + full*M3, with the
     combination matrices A/B/C as per-core input data, evaluated by tiny
     PE matmuls after transposing the [128, nchunk] accumulators.

3. Everything stays fp16 on the matmul path (fp8 fails: the ragged pooling
   amplifies per-token quantization noise by sqrt(count) ~ 32x).

Sharding: host cuts the sorted token axis at segment boundaries so every
segment belongs to exactly one core (no collectives); each core runs the
predictor on its own <=SLOTS segments and writes its slice of the output.
"""

import numpy as np

import concourse.mybir as mybir
import concourse.tile as tile
from concourse import bacc
from concourse import bass_utils
from concourse.masks import make_identity

# Problem shapes (hardcoded per contract).
T, E, H, C, O = 131072, 256, 512, 256, 32
S = 128            # num segments
N_CORES = 8
TOK = 1024         # tokens per chunk
NCPAD = 32         # padded chunk-count (accumulator columns)
MIN_SLOTS = 32     # baseline segments-per-core capacity
F32 = mybir.dt.float32
F32R = mybir.dt.float32r
F16 = mybir.dt.float16

_CACHE = {}


def _mm(nc, out, lhsT, rhs, start, stop, skip=True):
    nc.tensor.matmul(out, lhsT, rhs,
                     start=start, stop=stop, skip_group_check=skip)


def _build_nc(t_sh, SLOTS):
    assert t_sh % 128 == 0
    NC = (t_sh + TOK - 1) // TOK
    assert NC <= NCPAD

    nc = bacc.Bacc("TRN2", target_bir_lowering=False, debug=False,
                   num_devices=N_CORES)

    xt_d = nc.dram_tensor("xt", [E, t_sh], F16, kind="ExternalInput")
    w1_d = nc.dram_tensor("w1", [E, H], F16, kind="ExternalInput")
    w2_d = nc.dram_tensor("w2", [H, H], F16, kind="ExternalInput")
    w3_d = nc.dram_tensor("w3", [H, C], F32R, kind="ExternalInput")
    b1_d = nc.dram_tensor("b1", [H // 128, 128], F32, kind="ExternalInput")
    b2_d = nc.dram_tensor("b2", [H // 128, 128], F32, kind="ExternalInput")
    b3_d = nc.dram_tensor("b3", [1, C], F32, kind="ExternalInput")
    p1_d = nc.dram_tensor("p1", [C, H], F32R, kind="ExternalInput")
    p2_d = nc.dram_tensor("p2", [H, H], F32R, kind="ExternalInput")
    p3_d = nc.dram_tensor("p3", [H, O], F32R, kind="ExternalInput")
    pb1_d = nc.dram_tensor("pb1", [H // 128, 128], F32, kind="ExternalInput")
    pb2_d = nc.dram_tensor("pb2", [H // 128, 128], F32, kind="ExternalInput")
    pb3_d = nc.dram_tensor("pb3", [1, O], F32, kind="ExternalInput")
    cnt_d = nc.dram_tensor("cnt", [1, SLOTS], F32, kind="ExternalInput")
    iota_d = nc.dram_tensor("iota", [128, TOK], F16, kind="ExternalInput")
    bm1_d = nc.dram_tensor("bm1", [128, NCPAD], F32, kind="ExternalInput")
    bm2_d = nc.dram_tensor("bm2", [128, NCPAD], F32, kind="ExternalInput")
    am_d = nc.dram_tensor("am", [NCPAD, SLOTS], F32, kind="ExternalInput")
    bmm_d = nc.dram_tensor("bmm", [NCPAD, SLOTS], F32, kind="ExternalInput")
    cm_d = nc.dram_tensor("cm", [NCPAD, SLOTS], F32, kind="ExternalInput")
    out_d = nc.dram_tensor("pred", [SLOTS, O], F32, kind="ExternalOutput")

    EC = E // 128   # 2
    HC = H // 128   # 4
    CC = C // 128   # 2
    LT = mybir.AluOpType.is_lt
    MUL = mybir.AluOpType.mult

    with tile.TileContext(nc) as tc:
        with tc.tile_pool(name="wts", bufs=1) as wp, \
             tc.tile_pool(name="xt", bufs=3) as xtp, \
             tc.tile_pool(name="h1", bufs=2) as h1p, \
             tc.tile_pool(name="h2", bufs=2) as h2p, \
             tc.tile_pool(name="scr", bufs=2) as scp, \
             tc.tile_pool(name="small", bufs=1) as smp, \
             tc.tile_pool(name="ps1", bufs=2, space="PSUM") as ps1p, \
             tc.tile_pool(name="ps2", bufs=2, space="PSUM") as ps2p:

            # warm the ACT tanh table before the scalar queue fills with DMAs
            warm_sb = smp.tile([1, 1], F32, tag="warm", name="warm")
            nc.gpsimd.memset(warm_sb[:], 0.0)
            warm_o = smp.tile([1, 1], F32, tag="warmo", name="warmo")
            nc.scalar.activation(warm_o[:], warm_sb[:],
                                 mybir.ActivationFunctionType.Tanh)

            # ---- resident weights ----
            w1_t = wp.tile([128, EC, HC, 128], F16, tag="w1", name="w1t")
            nc.scalar.dma_start(
                w1_t[:], w1_d.ap().rearrange("(e p) (h q) -> p e h q",
                                             p=128, q=128))
            w1_sb = [[w1_t[:, e, h, :] for h in range(HC)] for e in range(EC)]
            w2_t = wp.tile([128, HC, HC, 128], F16, tag="w2", name="w2t")
            nc.scalar.dma_start(
                w2_t[:], w2_d.ap().rearrange("(k p) (h q) -> p k h q",
                                             p=128, q=128))
            w2_sb = [[w2_t[:, k, h, :] for h in range(HC)] for k in range(HC)]
            w3_t = wp.tile([128, HC, C], F32R, tag="w3", name="w3t")
            nc.gpsimd.dma_start(
                w3_t[:], w3_d.ap().rearrange("(k p) c -> p k c", p=128))
            w3_sb = [w3_t[:, k, :] for k in range(HC)]
            # biases / rows
            b1_sb = smp.tile([128, HC], F32, tag="b1", name="b1")
            nc.gpsimd.dma_start(b1_sb[:], b1_d.ap().rearrange("h p -> p h"))
            b2_sb = smp.tile([128, HC], F32, tag="b2", name="b2")
            nc.gpsimd.dma_start(b2_sb[:], b2_d.ap().rearrange("h p -> p h"))
            pb1_sb = smp.tile([128, HC], F32, tag="pb1", name="pb1")
            nc.gpsimd.dma_start(pb1_sb[:], pb1_d.ap().rearrange("h p -> p h"))
            pb2_sb = smp.tile([128, HC], F32, tag="pb2", name="pb2")
            nc.gpsimd.dma_start(pb2_sb[:], pb2_d.ap().rearrange("h p -> p h"))
            b3row = smp.tile([1, C], F32, tag="b3row", name="b3row")
            nc.gpsimd.dma_start(b3row[:], b3_d.ap())
            pb3row = smp.tile([1, O], F32, tag="pb3row", name="pb3row")
            nc.gpsimd.dma_start(pb3row[:], pb3_d.ap())
            cntrow = smp.tile([1, SLOTS], F32, tag="cntrow", name="cntrow")
            nc.gpsimd.dma_start(cntrow[:], cnt_d.ap())
            ones1 = smp.tile([1, SLOTS], F32, tag="ones1", name="ones1")
            nc.gpsimd.memset(ones1[:], 1.0)
            ident = smp.tile([128, 128], F32, tag="ident", name="ident")
            make_identity(nc, ident[:])
            iota_sb = smp.tile([128, TOK], F16, tag="iota", name="iota")
            nc.gpsimd.dma_start(iota_sb[:], iota_d.ap())
            bm1_sb = smp.tile([128, NCPAD], F32, tag="bm1", name="bm1")
            nc.gpsimd.dma_start(bm1_sb[:], bm1_d.ap())
            bm2_sb = smp.tile([128, NCPAD], F32, tag="bm2", name="bm2")
            nc.gpsimd.dma_start(bm2_sb[:], bm2_d.ap())
            am_sb = smp.tile([NCPAD, SLOTS], F32, tag="am", name="am")
            nc.gpsimd.dma_start(am_sb[:], am_d.ap())
            bmm_sb = smp.tile([NCPAD, SLOTS], F32, tag="bmm", name="bmm")
            nc.gpsimd.dma_start(bmm_sb[:], bmm_d.ap())
            cm_sb = smp.tile([NCPAD, SLOTS], F32, tag="cm", name="cm")
            nc.gpsimd.dma_start(cm_sb[:], cm_d.ap())

            p1_t = wp.tile([128, CC, HC, 128], F32R, tag="p1", name="p1t")
            nc.gpsimd.dma_start(
                p1_t[:], p1_d.ap().rearrange("(c p) (h q) -> p c h q",
                                             p=128, q=128))
            p1_sb = [[p1_t[:, c, h, :] for h in range(HC)] for c in range(CC)]
            p2_t = wp.tile([128, HC, HC, 128], F32R, tag="p2", name="p2t")
            nc.gpsimd.dma_start(
                p2_t[:], p2_d.ap().rearrange("(k p) (h q) -> p k h q",
                                             p=128, q=128))
            p2_sb = [[p2_t[:, k, h, :] for h in range(HC)] for k in range(HC)]
            p3_t = wp.tile([128, HC, O], F32R, tag="p3", name="p3t")
            nc.gpsimd.dma_start(
                p3_t[:], p3_d.ap().rearrange("(k p) o -> p k o", p=128))
            p3_sb = [p3_t[:, k, :] for k in range(HC)]

            # ---- per-(h,chunk) pooling accumulators (fp32) ----
            facc = smp.tile([128, HC, NCPAD], F32, tag="facc", name="facc")
            nc.gpsimd.memset(facc[:], 0.0)
            hacc1 = smp.tile([128, HC, NCPAD], F32, tag="hacc1", name="hacc1")
            nc.gpsimd.memset(hacc1[:], 0.0)
            hacc2 = smp.tile([128, HC, NCPAD], F32, tag="hacc2", name="hacc2")
            nc.gpsimd.memset(hacc2[:], 0.0)

            # ---- main token loop ----
            chunks = []
            base = 0
            while base < t_sh:
                ct = min(TOK, t_sh - base)
                chunks.append((base, ct))
                base += ct

            def halves(ct):
                if ct <= 512:
                    return [(0, ct)]
                return [(0, 512), (512, ct - 512)]

            def load_and_l1(ci, base, ct):
                xt_t = xtp.tile([128, EC, ct], F16, tag="xt", name="xt",
                                padded_shape=[128, EC, TOK])
                nc.sync.dma_start(
                    xt_t[:],
                    xt_d.ap()[:, base:base + ct]
                        .rearrange("(e p) t -> p e t", p=128))
                h1_t = h1p.tile([128, HC, ct], F16, tag="h1", name="h1",
                                padded_shape=[128, HC, TOK])
                for h in range(HC):
                    ps1 = ps1p.tile([128, ct], F32, tag="ps1", name="ps1",
                                    padded_shape=[128, TOK])
                    for (hb, hl) in halves(ct):
                        for e in range(EC):
                            _mm(nc, ps1[:, hb:hb + hl], w1_sb[e][h],
                                xt_t[:, e, hb:hb + hl],
                                start=(e == 0), stop=(e == EC - 1))
                    nc.scalar.activation(h1_t[:, h, :], ps1[:],
                                         mybir.ActivationFunctionType.Tanh,
                                         bias=b1_sb[:, 0 + h:h + 1])
                return h1_t

            def l2_and_seg(ci, base, ct, h1_t):
                h2_t = h2p.tile([128, HC, ct], F16, tag="h2", name="h2",
                                padded_shape=[128, HC, TOK])
                for h in range(HC):
                    ps2 = ps2p.tile([128, ct], F32, tag="ps2", name="ps2",
                                    padded_shape=[128, TOK])
                    for (hb, hl) in halves(ct):
                        for k in range(HC):
                            _mm(nc, ps2[:, hb:hb + hl], w2_sb[k][h],
                                h1_t[:, k, hb:hb + hl],
                                start=(k == 0), stop=(k == HC - 1))
                    nc.scalar.activation(h2_t[:, h, :], ps2[:],
                                         mybir.ActivationFunctionType.Tanh,
                                         bias=b2_sb[:, h:h + 1],
                                         accum_out=facc[:, h, ci:ci + 1])
                    # head sums before the (<=2) intra-chunk boundaries:
                    # (iota < b) * h2 summed along tokens, b is per-core data
                    for acc, bm in ((hacc1, bm1_sb), (hacc2, bm2_sb)):
                        sc = scp.tile([128, ct], F16, tag="scr", name="scr",
                                      bufs=4, padded_shape=[128, TOK])
                        nc.vector.scalar_tensor_tensor(
                            sc[:], iota_sb[:, :ct], bm[:, ci:ci + 1],
                            h2_t[:, h, :], LT, MUL,
                            accum_out=acc[:, h, ci:ci + 1])

            # software pipeline: L1(i+1) is emitted before L2/seg(i)
            pend = []
            for ci, (base, ct) in enumerate(chunks):
                pend.append((ci, base, ct, load_and_l1(ci, base, ct)))
                keep = 2 if ci < 2 else 1
                while len(pend) > keep:
                    l2_and_seg(*pend.pop(0))
            while pend:
                l2_and_seg(*pend.pop(0))

            # ---- assemble hsegT[h, s] from the chunk accumulators ----
            # transpose the [128, NCPAD] accumulators to [NCPAD, 128]
            accT = smp.tile([NCPAD, 3, HC, 128], F32, tag="accT", name="accT")
            for j, acc in enumerate((hacc1, hacc2, facc)):
                for h in range(HC):
                    tp = ps1p.tile([NCPAD, 128], F32, tag="ps1", name="tp")
                    nc.tensor.transpose(tp[:], acc[:, h, :], ident[:])
                    nc.vector.tensor_copy(accT[:, j, h, :], tp[:])

            # hsegT = head1 @ (M1-M2) + head2 @ (M2-M3) + full @ M3
            hsegT = smp.tile([128, HC, SLOTS], F32R, tag="hsegT",
                             name="hsegT")
            for h in range(HC):
                hs = ps2p.tile([128, SLOTS], F32, tag="ps2", name="hs")
                _mm(nc, hs[:], accT[:, 0, h, :], am_sb[:],
                    start=True, stop=False)
                _mm(nc, hs[:], accT[:, 1, h, :], bmm_sb[:],
                    start=False, stop=False)
                _mm(nc, hs[:], accT[:, 2, h, :], cm_sb[:],
                    start=False, stop=True)
                nc.vector.tensor_copy(hsegT[:, h, :], hs[:])

            # ---- encT[c, s] = W3.T @ hsegT + b3 x counts ----
            encT_sb = smp.tile([128, CC, SLOTS], F32R, tag="encT",
                               name="encT")
            for c in range(CC):
                ep = ps1p.tile([128, SLOTS], F32, tag="ps1", name="ep")
                nc.tensor.matmul(ep[:], b3row[:, c * 128:(c + 1) * 128],
                                 cntrow[:], start=True, stop=False,
                                 skip_group_check=True)
                for k in range(HC):
                    _mm(nc, ep[:], w3_sb[k][:, c * 128:(c + 1) * 128],
                        hsegT[:, k, :], start=False, stop=(k == HC - 1))
                nc.vector.tensor_copy(encT_sb[:, c, :], ep[:])

            # ---- predictor MLP on this core's own <=SLOTS segment rows ----
            q1_sb = smp.tile([128, HC, SLOTS], F32R, tag="q1", name="q1")
            for h in range(HC):
                pp1 = ps2p.tile([128, SLOTS], F32, tag="ps2", name="pp1")
                for c in range(CC):
                    _mm(nc, pp1[:], p1_sb[c][h], encT_sb[:, c, :],
                        start=(c == 0), stop=(c == CC - 1))
                nc.scalar.activation(q1_sb[:, h, :], pp1[:],
                                     mybir.ActivationFunctionType.Tanh,
                                     bias=pb1_sb[:, h:h + 1])
            q2_sb = smp.tile([128, HC, SLOTS], F32R, tag="q2", name="q2")
            for h in range(HC):
                pp2 = ps1p.tile([128, SLOTS], F32, tag="ps1", name="pp2")
                for k in range(HC):
                    _mm(nc, pp2[:], p2_sb[k][h], q1_sb[:, k, :],
                        start=(k == 0), stop=(k == HC - 1))
                nc.scalar.activation(q2_sb[:, h, :], pp2[:],
                                     mybir.ActivationFunctionType.Tanh,
                                     bias=pb2_sb[:, h:h + 1])

            # final: pred[slot, o] = q2.T @ P3 + pb3
            ppo = ps2p.tile([SLOTS, O], F32, tag="ps2", name="ppo")
            nc.tensor.matmul(ppo[:], ones1[:], pb3row[:],
                             start=True, stop=False, skip_group_check=True)
            for k in range(HC):
                _mm(nc, ppo[:], q2_sb[:, k, :], p3_sb[k],
                    start=False, stop=(k == HC - 1))
            pred_sb = smp.tile([SLOTS, O], F32, tag="pred", name="predsb")
            nc.vector.tensor_copy(pred_sb[:], ppo[:])
            nc.sync.dma_start(out_d.ap(), pred_sb[:])

    nc.compile()
    return nc


def kernel(words, seg_ids, W1, b1, W2, b2, W3, b3,
           P1, pb1, P2, pb2, P3, pb3, batch_size, alpha_iter, **_):
    words = np.asarray(words, dtype=np.float32)
    seg_ids = np.asarray(seg_ids).astype(np.int64)
    assert words.shape == (T, E), words.shape
    bs, ai = int(batch_size), int(alpha_iter)
    assert bs * ai == S

    # --- host-side index prep: cut the sorted token axis at segment
    # boundaries so each core owns whole segments ---
    counts = np.bincount(seg_ids, minlength=S)[:S]
    starts = np.concatenate([[0], np.cumsum(counts)])   # [S+1]
    cuts = [0]
    for c in range(1, N_CORES):
        tgt = c * T // N_CORES
        j = int(np.searchsorted(starts, tgt, side="left"))
        if j > 0 and tgt - starts[j - 1] < starts[j] - tgt:
            j -= 1
        cuts.append(int(starts[j]))
    cuts.append(T)
    lens = np.diff(cuts)
    t_sh = int(np.ceil(lens.max() / 128) * 128)

    seg_lo = [0] * N_CORES
    for c in range(N_CORES - 1, 0, -1):
        if lens[c] > 0:
            seg_lo[c] = int(seg_ids[cuts[c]])
        else:
            seg_lo[c] = S if c == N_CORES - 1 else seg_lo[c + 1]
    seg_hi = seg_lo[1:] + [S]
    slots_needed = max(seg_hi[c] - seg_lo[c] for c in range(N_CORES))
    SLOTS = min(128, max(MIN_SLOTS, ((slots_needed + 31) // 32) * 32))
    assert slots_needed <= SLOTS, (seg_lo, seg_hi)

    xt = np.ascontiguousarray(words.T.astype(np.float16))    # [E, T] fp16

    key = ("nc", t_sh, SLOTS)
    if key not in _CACHE:
        _CACHE[key] = _build_nc(t_sh, SLOTS)
    nc = _CACHE[key]

    NC = (t_sh + TOK - 1) // TOK
    iota = np.broadcast_to(np.arange(TOK, dtype=np.float16),
                           (128, TOK)).copy()

    common = {
        "w1": np.ascontiguousarray(W1, dtype=np.float16),
        "w2": np.ascontiguousarray(W2, dtype=np.float16),
        "w3": np.ascontiguousarray(W3, dtype=np.float32),
        "b1": np.ascontiguousarray(b1, dtype=np.float32).reshape(H // 128, 128),
        "b2": np.ascontiguousarray(b2, dtype=np.float32).reshape(H // 128, 128),
        "b3": np.ascontiguousarray(b3, dtype=np.float32).reshape(1, C),
        "p1": np.ascontiguousarray(P1, dtype=np.float32),
        "p2": np.ascontiguousarray(P2, dtype=np.float32),
        "p3": np.ascontiguousarray(P3, dtype=np.float32),
        "pb1": np.ascontiguousarray(pb1, dtype=np.float32).reshape(H // 128, 128),
        "pb2": np.ascontiguousarray(pb2, dtype=np.float32).reshape(H // 128, 128),
        "pb3": np.ascontiguousarray(pb3, dtype=np.float32).reshape(1, O),
        "iota": iota,
    }
    in_maps = []
    for c in range(N_CORES):
        lo, hi = cuts[c], cuts[c + 1]
        n = hi - lo
        xt_c = np.zeros((E, t_sh), dtype=np.float16)
        xt_c[:, :n] = xt[:, lo:hi]
        sl = seg_ids[lo:hi] - seg_lo[c]          # local slot per token
        nseg = seg_hi[c] - seg_lo[c]
        assert n == 0 or (sl.min() >= 0 and sl.max() < SLOTS)

        bm1 = np.zeros(NCPAD, dtype=np.float32)
        bm2 = np.zeros(NCPAD, dtype=np.float32)
        M1 = np.zeros((NCPAD, SLOTS), dtype=np.float32)
        M2 = np.zeros((NCPAD, SLOTS), dtype=np.float32)
        M3 = np.zeros((NCPAD, SLOTS), dtype=np.float32)
        base = 0
        ci = 0
        while base < t_sh:
            ct = min(TOK, t_sh - base)
            nn = min(max(n - base, 0), ct)       # valid tokens in chunk
            if nn > 0:
                ss = sl[base:base + nn]
                bs_pos = (np.nonzero(np.diff(ss))[0] + 1).tolist()
                assert len(bs_pos) <= 2, (c, ci, len(bs_pos))
                if len(bs_pos) == 0:
                    b1c, b2c = nn, nn
                    M1[ci, ss[0]] = 1.0
                elif len(bs_pos) == 1:
                    b1c, b2c = bs_pos[0], nn
                    M1[ci, ss[0]] = 1.0
                    M2[ci, ss[b1c]] = 1.0
                else:
                    assert nn == ct, "two boundaries + padding in one chunk"
                    b1c, b2c = bs_pos
                    M1[ci, ss[0]] = 1.0
                    M2[ci, ss[b1c]] = 1.0
                    M3[ci, ss[b2c]] = 1.0
                bm1[ci], bm2[ci] = b1c, b2c
            base += ct
            ci += 1

        cnt_c = np.zeros((1, SLOTS), dtype=np.float32)
        cnt_c[0, :nseg] = counts[seg_lo[c]:seg_hi[c]]
        in_maps.append({
            **common,
            "xt": xt_c,
            "cnt": cnt_c,
            "bm1": np.broadcast_to(bm1, (128, NCPAD)).copy(),
            "bm2": np.broadcast_to(bm2, (128, NCPAD)).copy(),
            "am": M1 - M2,
            "bmm": M2 - M3,
            "cm": np.ascontiguousarray(M3),
        })

    global _LAST_IN_MAPS
    _LAST_IN_MAPS = in_maps
    res = bass_utils.run_bass_kernel_spmd(nc, in_maps,
                                          core_ids=list(range(N_CORES)))
    pred = np.zeros((S, O), dtype=np.float32)
    for c in range(N_CORES):
        nseg = seg_hi[c] - seg_lo[c]
        if nseg > 0:
            pred[seg_lo[c]:seg_hi[c]] = res.results[c]["pred"][:nseg]
    return pred.reshape(bs, ai, O).astype(np.float32)


_LAST_IN_MAPS = None
